# revision 1
# baseline (speedup 1.0000x reference)
"""Trainium2 Bass kernel for nn_LNon_37460704756094 (embedding_lookup).

Math (reference):
    d   = (data - mean(data)) / std(data, ddof=1) * scalei
    s   = sigmoid(d); t = tanh(d)
    theta = interp(theta_lut, s * 119)   # theta_lut = linspace(-pi, pi, 120)
    velo  = interp(velo_lut, |t| * 119)  # velo_lut  = linspace(0, 3, 120)
    val = d * exp(velo * sin(theta)) + velo * cos(theta)
    out = (val - mean(val)) / std(val, ddof=1) * scaleo

Both LUTs are affine in the index, so linear interpolation collapses to an
affine map of the (continuous) index:
    theta = th0 + (th119 - th0) * s        (exact for an affine LUT)
    velo  = (v119 - v0) * |t|  (+ v0, asserted ~0)
cos(theta) = sin(theta + pi/2), so everything becomes Sigmoid/Tanh/Abs/Sin/
Exp/Square activations + a few vector ops. The affine coefficients are read
from the actual `params` input on the host at call time.

Distribution: batch-sharded over 8 cores (4 batches each = [128, 32768] f32
per core, SBUF-resident). Global mean/std for both normalizations via
per-partition accumulation -> partition_all_reduce -> 8-core AllReduce of a
[128, 2] stats buffer. HBM traffic is one 16 MiB read + one 16 MiB write
per core.
"""

import math

import numpy as np

import concourse.bacc as bacc
import concourse.bass as bass
import concourse.mybir as mybir
import concourse.tile as tile
from concourse.bass_utils import run_bass_kernel_spmd

N_CORES = 8
P = 128
B_FULL, C, H, W = 32, 64, 128, 128
PER_CORE = B_FULL // N_CORES * C * H * W          # 4,194,304
FREE = PER_CORE // P                              # 32,768
F = 1024                                          # tile free size
NT = FREE // F                                    # 32 tiles
N_TOTAL = B_FULL * C * H * W                      # 33,554,432

AF = mybir.ActivationFunctionType
ALU = mybir.AluOpType
AX = mybir.AxisListType
F32 = mybir.dt.float32

LAST_RESULT = None  # BassKernelResults of the most recent run (for test.py)

_KERNEL_CACHE = {}


def _build(consts, sim_mode=False):
    """Build the SPMD Bass program. `consts` = (th0, th_slope, v_slope)."""
    th0, th_slope, v_slope = consts
    halfpi = math.pi / 2.0

    nc = bacc.Bacc(None, num_devices=N_CORES)

    # Register the Sin biases as const APs (activation float biases are
    # looked up in nc.const_aps). Same pattern as Bass.__init__.
    for cv in (th0, th0 + halfpi):
        if (F32, cv) not in nc.const_aps.aps:
            t = nc.alloc_sbuf_tensor(f"const-f32-{cv}", [P, 1], F32)
            nc.gpsimd.memset(t.ap(), cv)
            nc.const_aps.aps[(F32, cv)] = t.ap()
    nc.all_engine_barrier()

    data_in = nc.dram_tensor("data", [P, FREE], F32, kind="ExternalInput")
    scal_in = nc.dram_tensor("scal", [P, 2], F32, kind="ExternalInput")
    out_dram = nc.dram_tensor("out", [P, FREE], F32, kind="ExternalOutput")

    groups = [list(range(N_CORES))]

    with tile.TileContext(nc) as tc:
        with (
            tc.tile_pool(name="big", bufs=1) as bigpool,
            tc.tile_pool(name="scr", bufs=3) as scr,
            tc.tile_pool(name="small", bufs=1) as smallpool,
            tc.tile_pool(name="psum", bufs=1, space="PSUM") as psumpool,
            tc.tile_pool(name="dram", bufs=1, space="DRAM") as dram,
        ):
            bigs = [bigpool.tile([P, F], F32, name=f"big{j}", tag=f"big{j}") for j in range(NT)]
            # per-tile partial stats: cols [0:NT) sum(x), [NT:2NT) sum(x^2),
            # [2NT:3NT) sum(val), [3NT:4NT) sum(val^2)
            statbuf = smallpool.tile([P, 4 * NT], F32, name="statbuf", tag="statbuf")
            # small scalars; phase A uses cols 0..15, phase B cols 16..31
            sm = smallpool.tile([P, 32], F32, name="sm", tag="sm")
            stA = smallpool.tile([P, 2], F32, name="stA", tag="stA")
            stB = smallpool.tile([P, 2], F32, name="stB", tag="stB")
            scal_all = smallpool.tile([P, 2], F32, name="scal_all", tag="scal_all")
            ones = smallpool.tile([P, P], F32, name="ones", tag="ones")
            psumA = psumpool.tile([P, 2], F32, name="psumA", tag="psumA")
            psumB = psumpool.tile([P, 2], F32, name="psumB", tag="psumB")

            cc_a_in = dram.tile([P, 2], F32, name="cc_a_in", tag="cc_a_in")
            cc_a_out = dram.tile([P, 2], F32, name="cc_a_out", tag="cc_a_out")
            cc_b_in = dram.tile([P, 2], F32, name="cc_b_in", tag="cc_b_in")
            cc_b_out = dram.tile([P, 2], F32, name="cc_b_out", tag="cc_b_out")

            # scalei / scaleo come pre-broadcast from the host as [128, 2]
            nc.gpsimd.dma_start(scal_all[:], scal_in[:])
            nc.vector.memset(ones[:], 1.0)

            # ---------------- Phase A: load + input stats ----------------
            for j in range(NT):
                sl = slice(j * F, (j + 1) * F)
                nc.sync.dma_start(bigs[j][:], data_in[:, sl])
                sq = scr.tile([P, F], F32, name="sq", tag="p")
                nc.scalar.activation(
                    sq[:], bigs[j][:], AF.Square,
                    accum_out=statbuf[:, NT + j : NT + j + 1],
                )
                nc.vector.reduce_sum(
                    statbuf[:, j : j + 1], bigs[j][:], axis=AX.X
                )

            nc.vector.reduce_sum(stA[:, 0:1], statbuf[:, 0:NT], axis=AX.X)
            nc.vector.reduce_sum(stA[:, 1:2], statbuf[:, NT : 2 * NT], axis=AX.X)

            # cross-core AllReduce of the [128, 2] per-partition partials
            nc.gpsimd.dma_start(cc_a_in[:], stA[:])
            if sim_mode:
                nc.gpsimd.dma_start(cc_a_out[:], cc_a_in[:])
            else:
                nc.gpsimd.collective_compute(
                    "AllReduce", ALU.add, replica_groups=groups,
                    ins=[cc_a_in.opt()], outs=[cc_a_out.opt()],
                )
            nc.gpsimd.dma_start(stA[:], cc_a_out[:])
            # ones.T @ stA: reduces across partitions AND broadcasts the
            # totals to every partition in one idle-PE matmul
            nc.tensor.matmul(psumA[:], ones[:], stA[:])
            nc.vector.tensor_copy(sm[:, 0:2], psumA[:])

            # a = scalei / std, b = -mean * a   (std unbiased, ddof=1)
            nc.vector.tensor_scalar_mul(sm[:, 2:3], sm[:, 0:1], 1.0 / N_TOTAL)   # mean
            nc.vector.tensor_mul(sm[:, 3:4], sm[:, 0:1], sm[:, 2:3])             # S1*mean
            nc.vector.tensor_sub(sm[:, 4:5], sm[:, 1:2], sm[:, 3:4])
            nc.vector.tensor_scalar_mul(sm[:, 5:6], sm[:, 4:5], 1.0 / (N_TOTAL - 1))
            nc.scalar.activation(sm[:, 6:7], sm[:, 5:6], AF.Sqrt)                # std
            nc.vector.reciprocal(sm[:, 7:8], sm[:, 6:7])                         # 1/std
            nc.vector.tensor_mul(sm[:, 8:9], sm[:, 7:8], scal_all[:, 0:1])      # a
            nc.vector.tensor_mul(sm[:, 9:10], sm[:, 2:3], sm[:, 8:9])
            nc.vector.tensor_scalar_mul(sm[:, 10:11], sm[:, 9:10], -1.0)         # b
            a_ap = sm[:, 8:9]
            b_ap = sm[:, 10:11]

            # ---------------- Phase B: elementwise chain + val stats -----
            for j in range(NT):
                d = bigs[j][:]
                s_ = scr.tile([P, F], F32, name="s", tag="s")
                t_ = scr.tile([P, F], F32, name="t", tag="t")
                u_ = scr.tile([P, F], F32, name="u", tag="u", bufs=2)
                T3 = scr.tile([P, F], F32, name="T3", tag="T3", bufs=2)
                sin_ = scr.tile([P, F], F32, name="sin", tag="sin")
                cos_ = scr.tile([P, F], F32, name="cos", tag="cos")
                p_ = scr.tile([P, F], F32, name="p", tag="p")

                nc.scalar.activation(s_[:], d, AF.Sigmoid, bias=b_ap, scale=a_ap)
                nc.scalar.activation(t_[:], d, AF.Tanh, bias=b_ap, scale=a_ap)
                nc.vector.tensor_scalar(
                    u_[:], d, a_ap, b_ap, op0=ALU.mult, op1=ALU.add
                )
                nc.scalar.activation(T3[:], t_[:], AF.Abs, scale=v_slope)
                nc.scalar.activation(sin_[:], s_[:], AF.Sin, bias=th0, scale=th_slope)
                nc.scalar.activation(
                    cos_[:], s_[:], AF.Sin, bias=th0 + halfpi, scale=th_slope
                )
                nc.vector.tensor_mul(p_[:], T3[:], sin_[:])
                nc.scalar.activation(sin_[:], p_[:], AF.Exp)                 # e
                nc.vector.tensor_mul(cos_[:], T3[:], cos_[:])                # q
                nc.vector.tensor_mul(u_[:], u_[:], sin_[:])                  # r = u*e
                nc.vector.tensor_add(d, u_[:], cos_[:])                      # val
                nc.scalar.activation(
                    t_[:], d, AF.Square,
                    accum_out=statbuf[:, 3 * NT + j : 3 * NT + j + 1],
                )
                nc.vector.reduce_sum(
                    statbuf[:, 2 * NT + j : 2 * NT + j + 1], d, axis=AX.X
                )

            nc.vector.reduce_sum(stB[:, 0:1], statbuf[:, 2 * NT : 3 * NT], axis=AX.X)
            nc.vector.reduce_sum(stB[:, 1:2], statbuf[:, 3 * NT : 4 * NT], axis=AX.X)

            nc.gpsimd.dma_start(cc_b_in[:], stB[:])
            if sim_mode:
                nc.gpsimd.dma_start(cc_b_out[:], cc_b_in[:])
            else:
                nc.gpsimd.collective_compute(
                    "AllReduce", ALU.add, replica_groups=groups,
                    ins=[cc_b_in.opt()], outs=[cc_b_out.opt()],
                )
            nc.gpsimd.dma_start(stB[:], cc_b_out[:])
            nc.tensor.matmul(psumB[:], ones[:], stB[:])
            nc.vector.tensor_copy(sm[:, 16:18], psumB[:])

            nc.vector.tensor_scalar_mul(sm[:, 18:19], sm[:, 16:17], 1.0 / N_TOTAL)
            nc.vector.tensor_mul(sm[:, 19:20], sm[:, 16:17], sm[:, 18:19])
            nc.vector.tensor_sub(sm[:, 20:21], sm[:, 17:18], sm[:, 19:20])
            nc.vector.tensor_scalar_mul(sm[:, 21:22], sm[:, 20:21], 1.0 / (N_TOTAL - 1))
            nc.scalar.activation(sm[:, 22:23], sm[:, 21:22], AF.Sqrt)
            nc.vector.reciprocal(sm[:, 23:24], sm[:, 22:23])
            nc.vector.tensor_mul(sm[:, 24:25], sm[:, 23:24], scal_all[:, 1:2])  # a2
            nc.vector.tensor_mul(sm[:, 25:26], sm[:, 18:19], sm[:, 24:25])
            nc.vector.tensor_scalar_mul(sm[:, 26:27], sm[:, 25:26], -1.0)        # b2
            a2_ap = sm[:, 24:25]
            b2_ap = sm[:, 26:27]

            # ---------------- Phase C: normalize + store -----------------
            for j in range(NT):
                sl = slice(j * F, (j + 1) * F)
                o_ = scr.tile([P, F], F32, name="o", tag="s")
                nc.vector.tensor_scalar(
                    o_[:], bigs[j][:], a2_ap, b2_ap, op0=ALU.mult, op1=ALU.add
                )
                nc.sync.dma_start(out_dram[:, sl], o_[:])

    nc.finalize()
    return nc


def kernel(data, params, scalei, scaleo):
    global LAST_RESULT
    data = np.ascontiguousarray(np.asarray(data, dtype=np.float32))
    params = np.asarray(params, dtype=np.float32)

    # Affine-LUT coefficients from the actual params input.
    th_lut = params[0, 0]
    v_lut = params[1, 0]
    npts = th_lut.shape[0]
    th0 = float(th_lut[0])
    th_slope = float(th_lut[npts - 1]) - th0
    v0 = float(v_lut[0])
    v_slope = float(v_lut[npts - 1]) - v0
    assert abs(v0) < 1e-6, f"velocity LUT must start at 0 (got {v0})"

    consts = (th0, th_slope, v_slope)
    nc = _KERNEL_CACHE.get(consts)
    if nc is None:
        nc = _build(consts)
        _KERNEL_CACHE[consts] = nc

    scal = np.tile(
        np.array(
            [[float(np.asarray(scalei).reshape(-1)[0]),
              float(np.asarray(scaleo).reshape(-1)[0])]],
            dtype=np.float32,
        ),
        (P, 1),
    )

    bpc = B_FULL // N_CORES
    in_maps = []
    for i in range(N_CORES):
        shard = np.ascontiguousarray(
            data[i * bpc : (i + 1) * bpc]
        ).reshape(P, FREE)
        in_maps.append({"data": shard, "scal": scal})

    res = run_bass_kernel_spmd(nc, in_maps, core_ids=list(range(N_CORES)))
    LAST_RESULT = res

    out = np.concatenate(
        [r["out"].reshape(bpc, C, H, W) for r in res.results], axis=0
    )
    return out



# revision 35
# speedup vs baseline: 1.2513x; 1.2513x over previous
"""Trainium2 Bass kernel for nn_LNon_37460704756094 (embedding_lookup).

Math (reference):
    d   = (data - mean(data)) / std(data, ddof=1) * scalei
    s   = sigmoid(d); t = tanh(d)
    theta = interp(theta_lut, s * 119)   # theta_lut = linspace(-pi, pi, 120)
    velo  = interp(velo_lut, |t| * 119)  # velo_lut  = linspace(0, 3, 120)
    val = d * exp(velo * sin(theta)) + velo * cos(theta)
    out = (val - mean(val)) / std(val, ddof=1) * scaleo

Key identities (exact for this LUT: th0 = -pi, th_slope = 2*pi):
    theta = 2*pi*sigmoid(u) - pi = pi * tanh(u/2)
    |tanh(u)| = tanh(|u|)
so with u = a*x + b (a = scalei/std, b = -mean*a):
    t12  = tanh(a/2*x + b/2)          [scalar: Tanh]
    sn   = sin(pi*t12)    = sin(theta)  [scalar: Sin]
    cs   = sin(pi*t12+pi/2) = cos(theta)[scalar: Sin]
    au   = |x - mean|                  [DVE: ts add/abs_max]
    T    = tanh(a*au)     = |tanh(u)|  [scalar: Tanh]
    m1   = T*sn                        [DVE: tt]
    e    = exp(vs*m1)                  [scalar: Exp]
    w    = (x - mean)*e                [Pool: stt]
    m2q  = (cs*vs)*T                   [Pool: stt]
    val  = a*w + m2q                   [Pool: stt, accum -> sum(val)]
    sumsq(val)                         [DVE: ttr]
Tanh/Sin/Sin/Tanh live in one activation table set (silu_and_others) and
Exp(+Square) in another, so batching activations per 4-tile group costs 2
table loads per group instead of ~3 per tile (1283 ns each).

Distribution: batch-sharded over 8 cores ([128, 32768] f32 per core). x is
cast to bf16 on ingest and kept SBUF-resident; val overwrites x in place.
Global stats via per-partition accumulation -> 8-core AllGather of [128,2]
partials -> local reduce -> ones-matmul partition broadcast.
"""

import math

import numpy as np

import concourse.bacc as bacc
import concourse.bass as bass
import concourse.mybir as mybir
import concourse.tile as tile
from concourse.bass_utils import run_bass_kernel_spmd

N_CORES = 8
P = 128
B_FULL, C, H, W = 32, 64, 128, 128
PER_CORE = B_FULL // N_CORES * C * H * W          # 4,194,304
FREE = PER_CORE // P                              # 32,768
F = 1024                                          # tile free size
NT = FREE // F                                    # 32 tiles
G = 4                                             # tiles per group
NG = NT // G                                      # 8 groups
GF = G * F                                        # 4096
N_TOTAL = B_FULL * C * H * W                      # 33,554,432

AF = mybir.ActivationFunctionType
ALU = mybir.AluOpType
AX = mybir.AxisListType
F32 = mybir.dt.float32
BF16 = mybir.dt.bfloat16
U16 = mybir.dt.uint16

PI = math.pi
HALFPI = math.pi / 2.0

LAST_RESULT = None  # BassKernelResults of the most recent run (for test.py)

_KERNEL_CACHE = {}

PROBE_FLAGS = set()  # debug-only feature toggles, see probe_hw.py


def _build(consts, n_cores=N_CORES):
    """Build the SPMD Bass program. `consts` = (v_slope,)."""
    (v_slope,) = consts
    single = n_cores == 1
    DT = F32 if "nobf16" in PROBE_FLAGS else BF16

    nc = bacc.Bacc(None, num_devices=n_cores)

    data_in = nc.dram_tensor("data", [P, FREE], F32, kind="ExternalInput")
    scal_in = nc.dram_tensor("scal", [P, 2], F32, kind="ExternalInput")
    out_dram = nc.dram_tensor("out", [P, FREE], F32, kind="ExternalOutput")

    groups = [list(range(N_CORES))]

    with tile.TileContext(nc) as tc:
        with (
            tc.tile_pool(name="big", bufs=1) as bigpool,
            tc.tile_pool(name="grp", bufs=1) as grp,
            tc.tile_pool(name="scr", bufs=3) as scr,
            tc.tile_pool(name="small", bufs=1) as smallpool,
            tc.tile_pool(name="psum", bufs=1, space="PSUM") as psumpool,
            tc.tile_pool(name="dram", bufs=1, space="DRAM") as dram,
        ):
            # x (then val, in place) for the whole core: [128, 32768] bf16
            xbuf = bigpool.tile([P, FREE], DT, name="xbuf", tag="xbuf")

            # stats: cols [0:8) sum(x) per group, [8:16) sum(x^2),
            # [16:24) sum(val), [24:32) sum(val^2)
            statbuf = smallpool.tile([P, 32], F32, name="statbuf", tag="statbuf")
            sm = smallpool.tile([P, 32], F32, name="sm", tag="sm")
            stA = smallpool.tile([P, 2], F32, name="stA", tag="stA")
            stB = smallpool.tile([P, 2], F32, name="stB", tag="stB")
            stS = smallpool.tile([P, 2], F32, name="stS", tag="stS")
            scal_all = smallpool.tile([P, 2], F32, name="scal_all", tag="scal_all")
            ones = smallpool.tile([P, P], F32, name="ones", tag="ones")
            zero_c = smallpool.tile([P, 1], F32, name="zero_c", tag="zero_c")
            halfpi_c = smallpool.tile([P, 1], F32, name="halfpi_c", tag="halfpi_c")
            psumA = psumpool.tile([P, 2], F32, name="psumA", tag="psumA")
            psumB = psumpool.tile([P, 2], F32, name="psumB", tag="psumB")

            cc_a_in = dram.tile([P, 2], F32, name="cc_a_in", tag="cc_a_in")
            cc_a_out = dram.tile([P, 2], F32, name="cc_a_out", tag="cc_a_out")
            cc_b_in = dram.tile([P, 2], F32, name="cc_b_in", tag="cc_b_in")
            cc_b_out = dram.tile([P, 2], F32, name="cc_b_out", tag="cc_b_out")

            nc.gpsimd.dma_start(scal_all[:], scal_in[:])
            nc.vector.memset(ones[:], 1.0)
            nc.gpsimd.memset(zero_c[:], 0.0)
            nc.gpsimd.memset(halfpi_c[:], HALFPI)
            zero = zero_c[:]

            # ---------------- Phase A: casting load + input stats ---------
            # gpsimd (SWDGE) DMA casts f32 HBM -> bf16 SBUF in the descriptor
            for g in range(NG):
                gsl = slice(g * GF, (g + 1) * GF)
                if "nobf16" in PROBE_FLAGS:
                    nc.sync.dma_start(xbuf[:, gsl], data_in[:, gsl])
                elif "nocastdma" in PROBE_FLAGS:
                    land = scr.tile([P, GF], F32, name="land", tag="land", bufs=2)
                    nc.sync.dma_start(land[:], data_in[:, gsl])
                    nc.vector.tensor_copy(xbuf[:, gsl], land[:])
                else:
                    nc.gpsimd.dma_start(xbuf[:, gsl], data_in[:, gsl])
                if "nosqaccum" in PROBE_FLAGS:
                    sqA = grp.tile([P, GF], F32, name="sqA", tag="sqf", bufs=2)
                    nc.vector.tensor_mul(sqA[:], xbuf[:, gsl], xbuf[:, gsl])
                    nc.vector.tensor_reduce(
                        statbuf[:, 8 + g : 9 + g], sqA[:], axis=AX.X, op=ALU.add
                    )
                else:
                    sqA = grp.tile([P, GF], DT, name="sqA", tag="sq", bufs=2)
                    nc.scalar.activation(
                        sqA[:], xbuf[:, gsl], AF.Square, bias=zero,
                        accum_out=statbuf[:, 8 + g : 9 + g],
                    )
                nc.vector.tensor_reduce(
                    statbuf[:, g : g + 1], xbuf[:, gsl], axis=AX.X, op=ALU.add
                )

            nc.vector.tensor_reduce(stA[:, 0:1], statbuf[:, 0:NG], axis=AX.X, op=ALU.add)
            nc.vector.tensor_reduce(stA[:, 1:2], statbuf[:, 8 : 8 + NG], axis=AX.X, op=ALU.add)

            # cross-core AllReduce of the [128, 2] per-partition partials
            nc.gpsimd.dma_start(cc_a_in[:], stA[:])
            if single:
                nc.gpsimd.dma_start(cc_a_out[:], cc_a_in[:])
            else:
                nc.gpsimd.collective_compute(
                    "AllReduce", ALU.add, replica_groups=groups,
                    ins=[cc_a_in.opt()], outs=[cc_a_out.opt()],
                )
            nc.gpsimd.dma_start(stS[:], cc_a_out[:])
            # ones.T @ stS: reduces across partitions AND broadcasts totals
            nc.tensor.matmul(psumA[:], ones[:], stS[:])
            nc.vector.tensor_copy(sm[:, 0:2], psumA[:])

            # a = scalei/std, ah = a/2, negmean = -mean, bh = -mean*a/2
            nc.vector.tensor_scalar_mul(sm[:, 2:3], sm[:, 0:1], 1.0 / N_TOTAL)  # mean
            nc.vector.tensor_mul(sm[:, 3:4], sm[:, 0:1], sm[:, 2:3])            # S1*mean
            nc.vector.tensor_sub(sm[:, 4:5], sm[:, 1:2], sm[:, 3:4])
            nc.vector.tensor_scalar_mul(sm[:, 5:6], sm[:, 4:5], 1.0 / (N_TOTAL - 1))
            nc.scalar.activation(sm[:, 6:7], sm[:, 5:6], AF.Sqrt, bias=zero)    # std
            nc.vector.reciprocal(sm[:, 7:8], sm[:, 6:7])                        # 1/std
            nc.vector.tensor_mul(sm[:, 8:9], sm[:, 7:8], scal_all[:, 0:1])      # a
            nc.vector.tensor_scalar_mul(sm[:, 9:10], sm[:, 8:9], 0.5)           # a/2
            nc.vector.tensor_scalar_mul(sm[:, 10:11], sm[:, 2:3], -1.0)         # -mean
            nc.vector.tensor_mul(sm[:, 11:12], sm[:, 10:11], sm[:, 9:10])       # -mean*a/2
            nc.vector.tensor_mul(sm[:, 12:13], sm[:, 10:11], sm[:, 8:9])        # -mean*a
            a_ap = sm[:, 8:9]
            ah_ap = sm[:, 9:10]
            bh_ap = sm[:, 11:12]
            b_ap = sm[:, 12:13]

            # ---------------- Phase B: elementwise chain + val stats ------
            # u = a*x + b; t12 = tanh(u/2); theta = pi*t12
            # T = tanh(|u|) = |tanh u|; e = exp(vs*T*sin);
            # val = u*e + (vs*cos)*T, written in place over x.
            # Per-512-chunk bn_stats -> (count, mean, M2) gives sum(val)
            # and sum(val^2) without DVE accum_out (broken on HW).
            NCH = GF // 512                      # bn_stats chunks per group
            bnbuf = smallpool.tile([P, NG * NCH, 6], F32, name="bnbuf", tag="bnbuf")
            valbuf = None
            if "noinplace" in PROBE_FLAGS:
                valbuf = bigpool.tile([P, FREE], DT, name="valbuf", tag="valbuf")
            for g in range(NG):
                gsl = slice(g * GF, (g + 1) * GF)
                x = xbuf[:, gsl]
                t12 = grp.tile([P, GF], DT, name="t12", tag="t12")
                t12a = grp.tile([P, GF], DT, name="t12a", tag="t12a")
                sn = grp.tile([P, GF], DT, name="sn", tag="sn")
                cs = grp.tile([P, GF], DT, name="cs", tag="cs")
                u_ = grp.tile([P, GF], DT, name="u", tag="u")
                au = grp.tile([P, GF], DT, name="au", tag="au")
                T_ = grp.tile([P, GF], DT, name="T", tag="T")
                m1 = grp.tile([P, GF], DT, name="m1", tag="m1")
                e_ = grp.tile([P, GF], DT, name="e", tag="e")
                w_ = grp.tile([P, GF], DT, name="w", tag="w")
                csv = grp.tile([P, GF], DT, name="csv", tag="sq", bufs=2)
                m2q = grp.tile([P, GF], DT, name="m2q", tag="m2q")

                # scalar chain (Tanh/Exp/Square share a set, Sin another)
                nc.scalar.activation(t12[:], x, AF.Tanh, bias=bh_ap, scale=ah_ap)
                # |t12| (clear sign bit) so cos(theta) = sin(pi/2 - pi*|t12|)
                # keeps the Sin input inside its valid range [-pi, pi]
                nc.vector.tensor_scalar(
                    t12a[:].bitcast(U16), t12[:].bitcast(U16), 0x7FFF, None,
                    op0=ALU.bitwise_and,
                )
                nc.scalar.activation(sn[:], t12[:], AF.Sin, bias=zero, scale=PI)
                nc.scalar.activation(cs[:], t12a[:], AF.Sin, bias=halfpi_c[:], scale=-PI)
                # u = a*x + b
                nc.vector.tensor_scalar(u_[:], x, a_ap, b_ap, op0=ALU.mult, op1=ALU.add)
                # au = |u|  (clear bf16 sign bit; abs_max isn't a legal ts op)
                nc.vector.tensor_scalar(
                    au[:].bitcast(U16), u_[:].bitcast(U16), 0x7FFF, None,
                    op0=ALU.bitwise_and,
                )
                nc.scalar.activation(T_[:], au[:], AF.Tanh, bias=zero)
                nc.vector.tensor_mul(m1[:], T_[:], sn[:])
                nc.scalar.activation(e_[:], m1[:], AF.Exp, bias=zero, scale=v_slope)
                # w = u * e
                nc.vector.tensor_mul(w_[:], u_[:], e_[:])
                # m2q = (cs * vs) * T
                nc.vector.tensor_scalar(csv[:], cs[:], float(v_slope), None, op0=ALU.mult)
                nc.vector.tensor_mul(m2q[:], csv[:], T_[:])
                valdst = x if valbuf is None else valbuf[:, gsl]
                # val = w + m2q  (in place over x)
                nc.vector.tensor_add(valdst, w_[:], m2q[:])
                # per-chunk stats of val
                for c in range(NCH):
                    csl = slice(g * GF + c * 512, g * GF + (c + 1) * 512)
                    vsl = (xbuf if valbuf is None else valbuf)[:, csl]
                    k = g * NCH + c
                    nc.vector.bn_stats(bnbuf[:, k : k + 1, :], vsl.unsqueeze(1))

            # fold bn chunk stats: sum = 256*(mu_e+mu_o);
            # sumsq = (M2_e+M2_o) + 256*(mu_e^2+mu_o^2)
            TCH = NG * NCH
            half = 512 // 2
            bs = smallpool.tile([P, 5, TCH], F32, name="bs", tag="bs")
            bnb = bnbuf[:]
            nc.vector.tensor_add(bs[:, 0:1, :].transpose([0, 2, 1]), bnb[:, :, 1:2], bnb[:, :, 4:5])
            nc.vector.tensor_mul(bs[:, 1:2, :].transpose([0, 2, 1]), bnb[:, :, 1:2], bnb[:, :, 1:2])
            nc.vector.tensor_mul(bs[:, 2:3, :].transpose([0, 2, 1]), bnb[:, :, 4:5], bnb[:, :, 4:5])
            nc.vector.tensor_add(bs[:, 3:4, :].transpose([0, 2, 1]), bnb[:, :, 2:3], bnb[:, :, 5:6])
            nc.vector.tensor_add(bs[:, 1:2, :], bs[:, 1:2, :], bs[:, 2:3, :])
            nc.vector.tensor_scalar(bs[:, 2:3, :], bs[:, 1:2, :], float(half), None, op0=ALU.mult)
            nc.vector.tensor_add(bs[:, 4:5, :], bs[:, 3:4, :], bs[:, 2:3, :])
            nc.vector.tensor_reduce(sm[:, 27:28], bs[:, 0:1, :], axis=AX.X, op=ALU.add)
            nc.vector.tensor_scalar(stB[:, 0:1], sm[:, 27:28], float(half), None, op0=ALU.mult)
            nc.vector.tensor_reduce(stB[:, 1:2], bs[:, 4:5, :], axis=AX.X, op=ALU.add)

            nc.gpsimd.dma_start(cc_b_in[:], stB[:])
            if single:
                nc.gpsimd.dma_start(cc_b_out[:], cc_b_in[:])
            else:
                nc.gpsimd.collective_compute(
                    "AllReduce", ALU.add, replica_groups=groups,
                    ins=[cc_b_in.opt()], outs=[cc_b_out.opt()],
                )
            nc.gpsimd.dma_start(stS[:], cc_b_out[:])
            nc.tensor.matmul(psumB[:], ones[:], stS[:])
            nc.vector.tensor_copy(sm[:, 16:18], psumB[:])

            nc.vector.tensor_scalar_mul(sm[:, 18:19], sm[:, 16:17], 1.0 / N_TOTAL)
            nc.vector.tensor_mul(sm[:, 19:20], sm[:, 16:17], sm[:, 18:19])
            nc.vector.tensor_sub(sm[:, 20:21], sm[:, 17:18], sm[:, 19:20])
            nc.vector.tensor_scalar_mul(sm[:, 21:22], sm[:, 20:21], 1.0 / (N_TOTAL - 1))
            nc.scalar.activation(sm[:, 22:23], sm[:, 21:22], AF.Sqrt, bias=zero)
            nc.vector.reciprocal(sm[:, 23:24], sm[:, 22:23])
            nc.vector.tensor_mul(sm[:, 24:25], sm[:, 23:24], scal_all[:, 1:2])  # a2
            nc.vector.tensor_mul(sm[:, 25:26], sm[:, 18:19], sm[:, 24:25])
            nc.vector.tensor_scalar_mul(sm[:, 26:27], sm[:, 25:26], -1.0)       # b2
            a2_ap = sm[:, 24:25]
            b2_ap = sm[:, 26:27]

            # ---------------- Phase C: normalize + store ------------------
            # out = a2*val + b2, alternating DVE ts / ScalarE Identity
            # (Identity is in every activation table set: no table load)
            vsrc = xbuf if valbuf is None else valbuf
            for j in range(NT):
                tsl = slice(j * F, (j + 1) * F)
                if j % 2 == 0:
                    o_ = scr.tile([P, F], F32, name="oD", tag="oD", bufs=2)
                    nc.vector.tensor_scalar(
                        o_[:], vsrc[:, tsl], a2_ap, b2_ap, op0=ALU.mult, op1=ALU.add
                    )
                else:
                    o_ = scr.tile([P, F], F32, name="oP", tag="oP", bufs=2)
                    nc.scalar.activation(
                        o_[:], vsrc[:, tsl], AF.Identity, bias=b2_ap, scale=a2_ap
                    )
                nc.sync.dma_start(out_dram[:, tsl], o_[:])

    nc.finalize()
    return nc


def kernel(data, params, scalei, scaleo):
    global LAST_RESULT
    data = np.ascontiguousarray(np.asarray(data, dtype=np.float32))
    params = np.asarray(params, dtype=np.float32)

    # Affine-LUT coefficients from the actual params input.
    th_lut = params[0, 0]
    v_lut = params[1, 0]
    npts = th_lut.shape[0]
    th0 = float(th_lut[0])
    th_slope = float(th_lut[npts - 1]) - th0
    v0 = float(v_lut[0])
    v_slope = float(v_lut[npts - 1]) - v0
    assert abs(v0) < 1e-6, f"velocity LUT must start at 0 (got {v0})"
    # theta = pi*tanh(u/2) identity requires th0 = -pi, th_slope = 2*pi
    assert abs(th0 + PI) < 1e-5, f"theta LUT must start at -pi (got {th0})"
    assert abs(th_slope - 2 * PI) < 1e-5, f"theta LUT slope must be 2*pi (got {th_slope})"

    consts = (v_slope,)
    nc = _KERNEL_CACHE.get(consts)
    if nc is None:
        nc = _build(consts)
        _KERNEL_CACHE[consts] = nc

    scal = np.tile(
        np.array(
            [[float(np.asarray(scalei).reshape(-1)[0]),
              float(np.asarray(scaleo).reshape(-1)[0])]],
            dtype=np.float32,
        ),
        (P, 1),
    )

    bpc = B_FULL // N_CORES
    in_maps = []
    for i in range(N_CORES):
        shard = np.ascontiguousarray(
            data[i * bpc : (i + 1) * bpc]
        ).reshape(P, FREE)
        in_maps.append({"data": shard, "scal": scal})

    res = run_bass_kernel_spmd(nc, in_maps, core_ids=list(range(N_CORES)))
    LAST_RESULT = res

    out = np.concatenate(
        [r["out"].reshape(bpc, C, H, W) for r in res.results], axis=0
    )
    return out


# revision 37
# speedup vs baseline: 1.4047x; 1.1227x over previous
"""Trainium2 Bass kernel for nn_LNon_37460704756094 (embedding_lookup).

Math (reference):
    d   = (data - mean(data)) / std(data, ddof=1) * scalei
    s   = sigmoid(d); t = tanh(d)
    theta = interp(theta_lut, s * 119)   # theta_lut = linspace(-pi, pi, 120)
    velo  = interp(velo_lut, |t| * 119)  # velo_lut  = linspace(0, 3, 120)
    val = d * exp(velo * sin(theta)) + velo * cos(theta)
    out = (val - mean(val)) / std(val, ddof=1) * scaleo

Key identities (exact for this LUT: th0 = -pi, th_slope = 2*pi):
    theta = 2*pi*sigmoid(u) - pi = pi * tanh(u/2)
    |tanh(u)| = tanh(|u|)
so with u = a*x + b (a = scalei/std, b = -mean*a):
    t12  = tanh(a/2*x + b/2)          [scalar: Tanh]
    sn   = sin(pi*t12)    = sin(theta)  [scalar: Sin]
    cs   = sin(pi*t12+pi/2) = cos(theta)[scalar: Sin]
    au   = |x - mean|                  [DVE: ts add/abs_max]
    T    = tanh(a*au)     = |tanh(u)|  [scalar: Tanh]
    m1   = T*sn                        [DVE: tt]
    e    = exp(vs*m1)                  [scalar: Exp]
    w    = (x - mean)*e                [Pool: stt]
    m2q  = (cs*vs)*T                   [Pool: stt]
    val  = a*w + m2q                   [Pool: stt, accum -> sum(val)]
    sumsq(val)                         [DVE: ttr]
Tanh/Sin/Sin/Tanh live in one activation table set (silu_and_others) and
Exp(+Square) in another, so batching activations per 4-tile group costs 2
table loads per group instead of ~3 per tile (1283 ns each).

Distribution: batch-sharded over 8 cores ([128, 32768] f32 per core). x is
cast to bf16 on ingest and kept SBUF-resident; val overwrites x in place.
Global stats via per-partition accumulation -> 8-core AllGather of [128,2]
partials -> local reduce -> ones-matmul partition broadcast.
"""

import math

import numpy as np

import concourse.bacc as bacc
import concourse.bass as bass
import concourse.mybir as mybir
import concourse.tile as tile
from concourse.bass_utils import run_bass_kernel_spmd

N_CORES = 8
P = 128
B_FULL, C, H, W = 32, 64, 128, 128
PER_CORE = B_FULL // N_CORES * C * H * W          # 4,194,304
FREE = PER_CORE // P                              # 32,768
F = 1024                                          # tile free size
NT = FREE // F                                    # 32 tiles
G = 4                                             # tiles per group
NG = NT // G                                      # 8 groups
GF = G * F                                        # 4096
N_TOTAL = B_FULL * C * H * W                      # 33,554,432

AF = mybir.ActivationFunctionType
ALU = mybir.AluOpType
AX = mybir.AxisListType
F32 = mybir.dt.float32
BF16 = mybir.dt.bfloat16
U16 = mybir.dt.uint16

PI = math.pi
HALFPI = math.pi / 2.0

LAST_RESULT = None  # BassKernelResults of the most recent run (for test.py)

_KERNEL_CACHE = {}

PROBE_FLAGS = set()  # debug-only feature toggles, see probe_hw.py


def _build(consts, n_cores=N_CORES):
    """Build the SPMD Bass program. `consts` = (v_slope,)."""
    (v_slope,) = consts
    single = n_cores == 1
    DT = F32 if "nobf16" in PROBE_FLAGS else BF16

    nc = bacc.Bacc(None, num_devices=n_cores)

    data_in = nc.dram_tensor("data", [P, FREE], F32, kind="ExternalInput")
    scal_in = nc.dram_tensor("scal", [P, 2], F32, kind="ExternalInput")
    out_dram = nc.dram_tensor("out", [P, FREE], F32, kind="ExternalOutput")

    groups = [list(range(N_CORES))]

    with tile.TileContext(nc) as tc:
        with (
            tc.tile_pool(name="big", bufs=1) as bigpool,
            tc.tile_pool(name="grp", bufs=1) as grp,
            tc.tile_pool(name="scr", bufs=3) as scr,
            tc.tile_pool(name="small", bufs=1) as smallpool,
            tc.tile_pool(name="psum", bufs=1, space="PSUM") as psumpool,
            tc.tile_pool(name="dram", bufs=1, space="DRAM") as dram,
        ):
            # x (then val, in place) for the whole core: [128, 32768] bf16
            xbuf = bigpool.tile([P, FREE], DT, name="xbuf", tag="xbuf")

            # stats: cols [0:8) sum(x) per group, [8:16) sum(x^2),
            # [16:24) sum(val), [24:32) sum(val^2)
            statbuf = smallpool.tile([P, 32], F32, name="statbuf", tag="statbuf")
            sm = smallpool.tile([P, 32], F32, name="sm", tag="sm")
            stA = smallpool.tile([P, 2], F32, name="stA", tag="stA")
            stB = smallpool.tile([P, 2], F32, name="stB", tag="stB")
            stS = smallpool.tile([P, 2], F32, name="stS", tag="stS")
            agbuf = smallpool.tile([P, 2, N_CORES], F32, name="agbuf", tag="agbuf")
            scal_all = smallpool.tile([P, 2], F32, name="scal_all", tag="scal_all")
            ones = smallpool.tile([P, P], F32, name="ones", tag="ones")
            zero_c = smallpool.tile([P, 1], F32, name="zero_c", tag="zero_c")
            halfpi_c = smallpool.tile([P, 1], F32, name="halfpi_c", tag="halfpi_c")
            psumA = psumpool.tile([P, 2], F32, name="psumA", tag="psumA")
            psumB = psumpool.tile([P, 2], F32, name="psumB", tag="psumB")

            cc_a_in = dram.tile([P, 2], F32, name="cc_a_in", tag="cc_a_in")
            cc_a_out = dram.tile([N_CORES, P, 2], F32, name="cc_a_out", tag="cc_a_out")
            cc_b_in = dram.tile([P, 2], F32, name="cc_b_in", tag="cc_b_in")
            cc_b_out = dram.tile([N_CORES, P, 2], F32, name="cc_b_out", tag="cc_b_out")

            nc.gpsimd.dma_start(scal_all[:], scal_in[:])
            nc.vector.memset(ones[:], 1.0)
            nc.gpsimd.memset(zero_c[:], 0.0)
            nc.gpsimd.memset(halfpi_c[:], HALFPI)
            zero = zero_c[:]

            # ---------------- Phase A: casting load + input stats ---------
            # gpsimd (SWDGE) DMA casts f32 HBM -> bf16 SBUF in the descriptor
            for g in range(NG):
                gsl = slice(g * GF, (g + 1) * GF)
                if "nobf16" in PROBE_FLAGS:
                    nc.sync.dma_start(xbuf[:, gsl], data_in[:, gsl])
                elif "nocastdma" in PROBE_FLAGS:
                    land = scr.tile([P, GF], F32, name="land", tag="land", bufs=2)
                    nc.sync.dma_start(land[:], data_in[:, gsl])
                    nc.vector.tensor_copy(xbuf[:, gsl], land[:])
                else:
                    nc.gpsimd.dma_start(xbuf[:, gsl], data_in[:, gsl])
                if "nosqaccum" in PROBE_FLAGS:
                    sqA = grp.tile([P, GF], F32, name="sqA", tag="sqf", bufs=2)
                    nc.vector.tensor_mul(sqA[:], xbuf[:, gsl], xbuf[:, gsl])
                    nc.vector.tensor_reduce(
                        statbuf[:, 8 + g : 9 + g], sqA[:], axis=AX.X, op=ALU.add
                    )
                else:
                    sqA = grp.tile([P, GF], DT, name="sqA", tag="sq", bufs=2)
                    nc.scalar.activation(
                        sqA[:], xbuf[:, gsl], AF.Square, bias=zero,
                        accum_out=statbuf[:, 8 + g : 9 + g],
                    )
                nc.vector.tensor_reduce(
                    statbuf[:, g : g + 1], xbuf[:, gsl], axis=AX.X, op=ALU.add
                )

            nc.vector.tensor_reduce(stA[:, 0:1], statbuf[:, 0:NG], axis=AX.X, op=ALU.add)
            nc.vector.tensor_reduce(stA[:, 1:2], statbuf[:, 8 : 8 + NG], axis=AX.X, op=ALU.add)

            # cross-core AllGather of [128, 2] partials + local reduce
            # (AllGather is ~1.9x cheaper than AllReduce in NRT)
            nc.gpsimd.dma_start(cc_a_in[:], stA[:])
            if single:
                nc.gpsimd.dma_start(cc_a_out[0:1], cc_a_in[:])
            else:
                nc.gpsimd.collective_compute(
                    "AllGather", ALU.bypass, replica_groups=groups,
                    ins=[cc_a_in.opt()], outs=[cc_a_out.opt()],
                )
            for r in range(N_CORES if not single else 1):
                nc.gpsimd.dma_start(agbuf[:, :, r : r + 1], cc_a_out[r : r + 1])
            if single:
                for r in range(1, N_CORES):
                    nc.gpsimd.dma_start(agbuf[:, :, r : r + 1], cc_a_out[0:1])
            if single:
                nc.vector.tensor_reduce(sm[:, 13:15], agbuf[:], axis=AX.X, op=ALU.add)
                nc.vector.tensor_scalar_mul(stS[:], sm[:, 13:15], 0.125)
            else:
                nc.vector.tensor_reduce(stS[:], agbuf[:], axis=AX.X, op=ALU.add)
            # ones.T @ stS: reduces across partitions AND broadcasts totals
            nc.tensor.matmul(psumA[:], ones[:], stS[:])
            nc.vector.tensor_copy(sm[:, 0:2], psumA[:])

            # a = scalei/std, ah = a/2, negmean = -mean, bh = -mean*a/2
            nc.vector.tensor_scalar_mul(sm[:, 2:3], sm[:, 0:1], 1.0 / N_TOTAL)  # mean
            nc.vector.tensor_mul(sm[:, 3:4], sm[:, 0:1], sm[:, 2:3])            # S1*mean
            nc.vector.tensor_sub(sm[:, 4:5], sm[:, 1:2], sm[:, 3:4])
            nc.vector.tensor_scalar_mul(sm[:, 5:6], sm[:, 4:5], 1.0 / (N_TOTAL - 1))
            nc.scalar.activation(sm[:, 6:7], sm[:, 5:6], AF.Sqrt, bias=zero)    # std
            nc.vector.reciprocal(sm[:, 7:8], sm[:, 6:7])                        # 1/std
            nc.vector.tensor_mul(sm[:, 8:9], sm[:, 7:8], scal_all[:, 0:1])      # a
            nc.vector.tensor_scalar_mul(sm[:, 9:10], sm[:, 8:9], 0.5)           # a/2
            nc.vector.tensor_scalar_mul(sm[:, 10:11], sm[:, 2:3], -1.0)         # -mean
            nc.vector.tensor_mul(sm[:, 11:12], sm[:, 10:11], sm[:, 9:10])       # -mean*a/2
            nc.vector.tensor_mul(sm[:, 12:13], sm[:, 10:11], sm[:, 8:9])        # -mean*a
            a_ap = sm[:, 8:9]
            ah_ap = sm[:, 9:10]
            bh_ap = sm[:, 11:12]
            b_ap = sm[:, 12:13]

            # ---------------- Phase B: elementwise chain + val stats ------
            # u = a*x + b; t12 = tanh(u/2); theta = pi*t12
            # T = tanh(|u|) = |tanh u|; e = exp(vs*T*sin);
            # val = u*e + (vs*cos)*T, written in place over x.
            # Per-512-chunk bn_stats -> (count, mean, M2) gives sum(val)
            # and sum(val^2) without DVE accum_out (broken on HW).
            NCH = GF // 512                      # bn_stats chunks per group
            bnbuf = smallpool.tile([P, NG * NCH, 6], F32, name="bnbuf", tag="bnbuf")
            valbuf = None
            if "noinplace" in PROBE_FLAGS:
                valbuf = bigpool.tile([P, FREE], DT, name="valbuf", tag="valbuf")
            for g in range(NG):
                gsl = slice(g * GF, (g + 1) * GF)
                x = xbuf[:, gsl]
                t12 = grp.tile([P, GF], DT, name="t12", tag="t12")
                t12a = grp.tile([P, GF], DT, name="t12a", tag="t12a")
                sn = grp.tile([P, GF], DT, name="sn", tag="sn")
                cs = grp.tile([P, GF], DT, name="cs", tag="cs")
                u_ = grp.tile([P, GF], DT, name="u", tag="u")
                au = grp.tile([P, GF], DT, name="au", tag="au")
                T_ = grp.tile([P, GF], DT, name="T", tag="T")
                m1 = grp.tile([P, GF], DT, name="m1", tag="m1")
                e_ = grp.tile([P, GF], DT, name="e", tag="e")
                w_ = grp.tile([P, GF], DT, name="w", tag="w")
                csv = grp.tile([P, GF], DT, name="csv", tag="sq", bufs=2)
                m2q = grp.tile([P, GF], DT, name="m2q", tag="m2q")

                # scalar chain (Tanh/Exp/Square share a set, Sin another)
                # u = a*x + b
                nc.vector.tensor_scalar(u_[:], x, a_ap, b_ap, op0=ALU.mult, op1=ALU.add)
                # au = |u|  (clear bf16 sign bit; abs_max isn't a legal ts op)
                nc.vector.tensor_scalar(
                    au[:].bitcast(U16), u_[:].bitcast(U16), 0x7FFF, None,
                    op0=ALU.bitwise_and,
                )
                # scalar order [Tanh,Tanh][Sin,Sin][Exp]: Tanh/Exp share an
                # activation table set, Sin needs another -> 2 loads/group
                nc.scalar.activation(t12[:], x, AF.Tanh, bias=bh_ap, scale=ah_ap)
                nc.scalar.activation(T_[:], au[:], AF.Tanh, bias=zero)
                # |t12| (clear sign bit) so cos(theta) = sin(pi/2 - pi*|t12|)
                # keeps the Sin input inside its valid range [-pi, pi]
                nc.vector.tensor_scalar(
                    t12a[:].bitcast(U16), t12[:].bitcast(U16), 0x7FFF, None,
                    op0=ALU.bitwise_and,
                )
                nc.scalar.activation(sn[:], t12[:], AF.Sin, bias=zero, scale=PI)
                nc.scalar.activation(cs[:], t12a[:], AF.Sin, bias=halfpi_c[:], scale=-PI)
                nc.vector.tensor_mul(m1[:], T_[:], sn[:])
                nc.scalar.activation(e_[:], m1[:], AF.Exp, bias=zero, scale=v_slope)
                # w = u * e
                nc.vector.tensor_mul(w_[:], u_[:], e_[:])
                # m2q = (cs * vs) * T
                nc.vector.tensor_scalar(csv[:], cs[:], float(v_slope), None, op0=ALU.mult)
                nc.vector.tensor_mul(m2q[:], csv[:], T_[:])
                valdst = x if valbuf is None else valbuf[:, gsl]
                # val = w + m2q  (in place over x)
                nc.vector.tensor_add(valdst, w_[:], m2q[:])
                # per-chunk stats of val
                for c in range(NCH):
                    csl = slice(g * GF + c * 512, g * GF + (c + 1) * 512)
                    vsl = (xbuf if valbuf is None else valbuf)[:, csl]
                    k = g * NCH + c
                    nc.vector.bn_stats(bnbuf[:, k : k + 1, :], vsl.unsqueeze(1))

            # fold bn chunk stats: sum = 256*(mu_e+mu_o);
            # sumsq = (M2_e+M2_o) + 256*(mu_e^2+mu_o^2)
            TCH = NG * NCH
            half = 512 // 2
            bs = smallpool.tile([P, 5, TCH], F32, name="bs", tag="bs")
            bnb = bnbuf[:]
            nc.vector.tensor_add(bs[:, 0:1, :].transpose([0, 2, 1]), bnb[:, :, 1:2], bnb[:, :, 4:5])
            nc.vector.tensor_mul(bs[:, 1:2, :].transpose([0, 2, 1]), bnb[:, :, 1:2], bnb[:, :, 1:2])
            nc.vector.tensor_mul(bs[:, 2:3, :].transpose([0, 2, 1]), bnb[:, :, 4:5], bnb[:, :, 4:5])
            nc.vector.tensor_add(bs[:, 3:4, :].transpose([0, 2, 1]), bnb[:, :, 2:3], bnb[:, :, 5:6])
            nc.vector.tensor_add(bs[:, 1:2, :], bs[:, 1:2, :], bs[:, 2:3, :])
            nc.vector.tensor_scalar(bs[:, 2:3, :], bs[:, 1:2, :], float(half), None, op0=ALU.mult)
            nc.vector.tensor_add(bs[:, 4:5, :], bs[:, 3:4, :], bs[:, 2:3, :])
            nc.vector.tensor_reduce(sm[:, 27:28], bs[:, 0:1, :], axis=AX.X, op=ALU.add)
            nc.vector.tensor_scalar(stB[:, 0:1], sm[:, 27:28], float(half), None, op0=ALU.mult)
            nc.vector.tensor_reduce(stB[:, 1:2], bs[:, 4:5, :], axis=AX.X, op=ALU.add)

            nc.gpsimd.dma_start(cc_b_in[:], stB[:])
            if single:
                nc.gpsimd.dma_start(cc_b_out[0:1], cc_b_in[:])
            else:
                nc.gpsimd.collective_compute(
                    "AllGather", ALU.bypass, replica_groups=groups,
                    ins=[cc_b_in.opt()], outs=[cc_b_out.opt()],
                )
            for r in range(N_CORES if not single else 1):
                nc.gpsimd.dma_start(agbuf[:, :, r : r + 1], cc_b_out[r : r + 1])
            if single:
                for r in range(1, N_CORES):
                    nc.gpsimd.dma_start(agbuf[:, :, r : r + 1], cc_b_out[0:1])
            if single:
                nc.vector.tensor_reduce(sm[:, 30:32], agbuf[:], axis=AX.X, op=ALU.add)
                nc.vector.tensor_scalar_mul(stS[:], sm[:, 30:32], 0.125)
            else:
                nc.vector.tensor_reduce(stS[:], agbuf[:], axis=AX.X, op=ALU.add)
            nc.tensor.matmul(psumB[:], ones[:], stS[:])
            nc.vector.tensor_copy(sm[:, 16:18], psumB[:])

            nc.vector.tensor_scalar_mul(sm[:, 18:19], sm[:, 16:17], 1.0 / N_TOTAL)
            nc.vector.tensor_mul(sm[:, 19:20], sm[:, 16:17], sm[:, 18:19])
            nc.vector.tensor_sub(sm[:, 20:21], sm[:, 17:18], sm[:, 19:20])
            nc.vector.tensor_scalar_mul(sm[:, 21:22], sm[:, 20:21], 1.0 / (N_TOTAL - 1))
            nc.scalar.activation(sm[:, 22:23], sm[:, 21:22], AF.Sqrt, bias=zero)
            nc.vector.reciprocal(sm[:, 23:24], sm[:, 22:23])
            nc.vector.tensor_mul(sm[:, 24:25], sm[:, 23:24], scal_all[:, 1:2])  # a2
            nc.vector.tensor_mul(sm[:, 25:26], sm[:, 18:19], sm[:, 24:25])
            nc.vector.tensor_scalar_mul(sm[:, 26:27], sm[:, 25:26], -1.0)       # b2
            a2_ap = sm[:, 24:25]
            b2_ap = sm[:, 26:27]

            # ---------------- Phase C: normalize + store ------------------
            # out = a2*val + b2, alternating DVE ts / ScalarE Identity
            # (Identity is in every activation table set: no table load)
            vsrc = xbuf if valbuf is None else valbuf
            for j in range(NT):
                tsl = slice(j * F, (j + 1) * F)
                if j % 2 == 0:
                    o_ = scr.tile([P, F], F32, name="oD", tag="oD", bufs=2)
                    nc.vector.tensor_scalar(
                        o_[:], vsrc[:, tsl], a2_ap, b2_ap, op0=ALU.mult, op1=ALU.add
                    )
                else:
                    o_ = scr.tile([P, F], F32, name="oP", tag="oP", bufs=2)
                    nc.scalar.activation(
                        o_[:], vsrc[:, tsl], AF.Identity, bias=b2_ap, scale=a2_ap
                    )
                nc.sync.dma_start(out_dram[:, tsl], o_[:])

    nc.finalize()
    return nc


def kernel(data, params, scalei, scaleo):
    global LAST_RESULT
    data = np.ascontiguousarray(np.asarray(data, dtype=np.float32))
    params = np.asarray(params, dtype=np.float32)

    # Affine-LUT coefficients from the actual params input.
    th_lut = params[0, 0]
    v_lut = params[1, 0]
    npts = th_lut.shape[0]
    th0 = float(th_lut[0])
    th_slope = float(th_lut[npts - 1]) - th0
    v0 = float(v_lut[0])
    v_slope = float(v_lut[npts - 1]) - v0
    assert abs(v0) < 1e-6, f"velocity LUT must start at 0 (got {v0})"
    # theta = pi*tanh(u/2) identity requires th0 = -pi, th_slope = 2*pi
    assert abs(th0 + PI) < 1e-5, f"theta LUT must start at -pi (got {th0})"
    assert abs(th_slope - 2 * PI) < 1e-5, f"theta LUT slope must be 2*pi (got {th_slope})"

    consts = (v_slope,)
    nc = _KERNEL_CACHE.get(consts)
    if nc is None:
        nc = _build(consts)
        _KERNEL_CACHE[consts] = nc

    scal = np.tile(
        np.array(
            [[float(np.asarray(scalei).reshape(-1)[0]),
              float(np.asarray(scaleo).reshape(-1)[0])]],
            dtype=np.float32,
        ),
        (P, 1),
    )

    bpc = B_FULL // N_CORES
    in_maps = []
    for i in range(N_CORES):
        shard = np.ascontiguousarray(
            data[i * bpc : (i + 1) * bpc]
        ).reshape(P, FREE)
        in_maps.append({"data": shard, "scal": scal})

    res = run_bass_kernel_spmd(nc, in_maps, core_ids=list(range(N_CORES)))
    LAST_RESULT = res

    out = np.concatenate(
        [r["out"].reshape(bpc, C, H, W) for r in res.results], axis=0
    )
    return out
